# revision 8
# baseline (speedup 1.0000x reference)
"""Trainium2 Bass kernel for nn_DiscreteStateTransition (NRI-style GNN message passing).

Reference computation (per batch b, time t):
  inputs[o]   = concat(x[b,o,t,:56], forward_probs[b,o,t,:8])          # [8, 64]
  pre_msg[e]  = concat(inputs[recv(e)], inputs[send(e)])               # [56, 128]
  h1          = relu(pre_msg @ W1 + b1)                                # [56, 512]
  msg         = relu(h1 @ W2 + b2)                                     # [56, 512]
  agg[o]      = sum over edges e with recv(e)==o of msg[e]             # [8, 512]
  out[o]      = concat(inputs[o], agg[o]) @ Wn + bn                    # [8, 64]

Restructuring for speed (vs the straightforward per-edge fp32r version):
  * Layer 1 decomposes per-node: h1[r,s] = relu(A[r] + B[s] + b1) with
    A = inputs @ W1[:64], B = inputs @ W1[64:] -- 7x fewer matmul columns.
    The per-edge combine runs on the DVE 4x all-bf16 fast path.
  * Layer 2 (the FLOP bulk) runs in bf16 (single-pass PE rate; rel err ~6e-3
    vs the 2e-2 budget).
  * The edge->node aggregation is a chain of fused (relu)+accumulate
    tensor_scalar ops on flat bf16 slices instead of strided reductions
    (TensorReduce has no DVE fast path).

Sharding: data-parallel over (B=4) x (T-halves=2) -> 8 cores. Each core owns one
(b, t-half) slice: [8 objects, 256 timesteps]. Weights replicated.

On-chip layout is feature-major: h features on SBUF partitions (one 128-row
block per "f" index), and the elementwise tensors store columns f-interleaved
as (edge, f, t) / (node, f, t) so the accumulation chains are flat 1-d slices
(walrus limits TensorScalarPtr to 2 free dims). h1's matmul copy (hdr) is
f-major to serve as the moving operand. Edges are ordered send-major
(s, rho, t) so per-recv sums have regular slices.
"""

import contextlib

import numpy as np
import ml_dtypes

import concourse.bacc as bacc
import concourse.mybir as mybir
import concourse.tile as tile
from concourse.bass_utils import run_bass_kernel_spmd
from concourse.masks import make_identity

F32 = mybir.dt.float32
BF = mybir.dt.bfloat16
MM_DT = mybir.dt.float32r   # fp32r: full-rate single-pass fp32 matmul

# Problem constants (hardcoded per the harness contract).
B, O, T = 4, 8, 512
D = 64            # node feature size (56 + 8)
E = 56            # directed edges = O*(O-1)
H = 512           # msg hidden/out size
KK = 64           # K*K output features
TC = 256          # timesteps per core
TB = 32           # timesteps per chunk
NCHUNK = TC // TB
CE = E * TB       # edge cols per chunk (1792), order (s, rho, t)
NN = O * TB       # node cols per chunk (256)


def build_nc(mm_dt=MM_DT, repeat=1):
    """Build the per-core Bass program (same program on all 8 cores)."""
    nc = bacc.Bacc("TRN2", target_bir_lowering=False, debug=False)

    xs = nc.dram_tensor("xs", [O, TC, 56], F32, kind="ExternalInput").ap()
    fps = nc.dram_tensor("fps", [O, TC, 8], F32, kind="ExternalInput").ap()
    w1 = nc.dram_tensor("w1", [2 * D, H], F32, kind="ExternalInput").ap()
    b1 = nc.dram_tensor("b1", [H], F32, kind="ExternalInput").ap()
    w2b = nc.dram_tensor("w2b", [H, H], BF, kind="ExternalInput").ap()
    b2 = nc.dram_tensor("b2", [H], F32, kind="ExternalInput").ap()
    wnin = nc.dram_tensor("wnin", [D, KK], F32, kind="ExternalInput").ap()
    wna = nc.dram_tensor("wna", [H, KK], BF, kind="ExternalInput").ap()
    bn = nc.dram_tensor("bn", [KK], F32, kind="ExternalInput").ap()
    out = nc.dram_tensor("out", [O, TC, KK], F32, kind="ExternalOutput").ap()

    AF = mybir.ActivationFunctionType
    ALU = mybir.AluOpType
    MD = mm_dt

    with tile.TileContext(nc) as tc:
        with (
            tc.tile_pool(name="const", bufs=1) as const,
            tc.tile_pool(name="inp", bufs=3) as inp_pool,
            tc.tile_pool(name="abp", bufs=2) as ab_pool,     # A'/B' bf16
            tc.tile_pool(name="up", bufs=2) as u_pool,       # u = A'+B' bf16
            tc.tile_pool(name="hdrp", bufs=2) as hdr_pool,   # h1 bf16 (f-major)
            tc.tile_pool(name="mstp", bufs=2) as mst_pool,   # msg bf16
            tc.tile_pool(name="aggp", bufs=2) as agg_pool,   # agg bf16
            tc.tile_pool(name="netp", bufs=2) as net_pool,
            tc.tile_pool(name="orm", bufs=4) as orm_pool,
            tc.tile_pool(name="abps", bufs=2, space="PSUM") as abps,
            tc.tile_pool(name="zps", bufs=4, space="PSUM") as zps,
            tc.tile_pool(name="opps", bufs=2, space="PSUM") as opps,
        ):
            # ---- constants / weights ----
            ident = const.tile([128, 128], F32)
            make_identity(nc, ident)

            w1s = const.tile([128, H], F32)
            nc.gpsimd.dma_start(w1s[:], w1)
            w1a = const.tile([64, H], MD)          # W1 recv half
            nc.scalar.copy(w1a[:], w1s[0:64, :])
            w1b = const.tile([64, H], MD)          # W1 send half
            nc.scalar.copy(w1b[:], w1s[64:128, :])

            w2t = const.tile([128, 4, H], BF)      # W2 bf16
            for k in range(4):
                nc.gpsimd.dma_start(w2t[:, k, :], w2b[k * 128:(k + 1) * 128, :])

            wnis = const.tile([64, KK], F32)
            nc.gpsimd.dma_start(wnis[:], wnin)
            wnit = const.tile([64, KK], MD)        # Wn input rows
            nc.scalar.copy(wnit[:], wnis[:])
            wnat = const.tile([128, 4, KK], BF)    # Wn agg rows (bf16)
            for k in range(4):
                nc.gpsimd.dma_start(wnat[:, k, :], wna[k * 128:(k + 1) * 128, :])

            b1t = const.tile([128, 4], F32)
            nc.gpsimd.dma_start(b1t[:], b1.rearrange("(f p) -> p f", p=128))
            b2t = const.tile([128, 4], F32)
            nc.gpsimd.dma_start(b2t[:], b2.rearrange("(f p) -> p f", p=128))
            bnt = const.tile([64, 1], F32)
            nc.gpsimd.dma_start(bnt[:], bn.unsqueeze(1))

            # ---- load node features, transpose to feature-major ----
            inputsT = const.tile([64, O * TC], MD)
            for th in range(2):
                for o in range(O):
                    rm = inp_pool.tile([128, 64], F32, name=f"rm{th}_{o}", tag="rm")
                    nc.sync.dma_start(rm[:, 0:56], xs[o, th * 128:(th + 1) * 128, :])
                    nc.sync.dma_start(rm[:, 56:64], fps[o, th * 128:(th + 1) * 128, :])
                    tp = opps.tile([64, 128], F32, name="tp", tag="op")
                    nc.tensor.transpose(tp[:], rm[:], ident[:])
                    cb0 = o * TC + th * 128
                    nc.scalar.copy(inputsT[:, cb0:cb0 + 128], tp[:])

            inT = inputsT.rearrange("p (o t) -> p o t", o=O)

            loop_ctx = (tc.For_i(0, repeat, 1,
                                 hint_engines=(mybir.EngineType.PE,))
                        if repeat > 1 else contextlib.nullcontext())
            with loop_ctx:
                st = {}   # per-chunk live tiles

                def emit_ab(c):
                    """A' = bf16(inc@W1a + b1), B' = bf16(inc@W1b). PE + Act.

                    Ap/Bp layout: [128, node, f, t] (f-interleaved columns).
                    """
                    inc = inT[:, :, c * TB:(c + 1) * TB]     # [64, 8, TB]
                    Ap = ab_pool.tile([128, O, 4, TB], BF, name="Ap", tag="Ap")
                    Bp = ab_pool.tile([128, O, 4, TB], BF, name="Bp", tag="Bp")
                    for half in range(2):
                        af = abps.tile([128, 2, NN], F32, name="af", tag="ab")
                        for i in range(2):
                            f = half * 2 + i
                            nc.tensor.matmul(af[:, i, :],
                                             w1a[:, f * 128:(f + 1) * 128], inc,
                                             start=True, stop=True)
                        for i in range(2):
                            f = half * 2 + i
                            nc.scalar.activation(
                                Ap[:, :, f, :],
                                af[:, i, :].rearrange("p (o t) -> p o t", t=TB),
                                AF.Identity, bias=b1t[:, f:f + 1])
                    for half in range(2):
                        bf_ = abps.tile([128, 2, NN], F32, name="bf", tag="ab")
                        for i in range(2):
                            f = half * 2 + i
                            nc.tensor.matmul(bf_[:, i, :],
                                             w1b[:, f * 128:(f + 1) * 128], inc,
                                             start=True, stop=True)
                        for i in range(2):
                            f = half * 2 + i
                            nc.scalar.copy(
                                Bp[:, :, f, :],
                                bf_[:, i, :].rearrange("p (o t) -> p o t", t=TB))
                    st[c] = {"Ap": Ap, "Bp": Bp, "inc": inc}

                def emit_u(c):
                    """u[s, rho, f, t] = A'[r(rho)] + B'[s] on DVE (4x bf16)."""
                    s_ = st[c]
                    u = u_pool.tile([128, E, 4, TB], BF, name="u", tag="u")
                    uf = u.rearrange("p e f t -> p (e f t)")
                    Af = s_["Ap"].rearrange("p o f t -> p (o f t)")
                    Bf = s_["Bp"].rearrange("p o f t -> p (o f t)")
                    W = 4 * TB   # cols per node/edge slot
                    for s in range(O):
                        bb = Bf[:, s * W:(s + 1) * W].unsqueeze(1)
                        if s > 0:      # rho < s  ->  r = rho
                            nc.vector.scalar_tensor_tensor(
                                out=uf[:, (s * 7) * W:(s * 7 + s) * W],
                                in0=Af[:, 0:s * W], scalar=0.0,
                                in1=bb.broadcast_to([128, s, W]),
                                op0=ALU.bypass, op1=ALU.add)
                        if s < O - 1:  # rho >= s ->  r = rho + 1
                            nc.vector.scalar_tensor_tensor(
                                out=uf[:, (s * 7 + s) * W:(s * 7 + 7) * W],
                                in0=Af[:, (s + 1) * W:O * W], scalar=0.0,
                                in1=bb.broadcast_to([128, O - 1 - s, W]),
                                op0=ALU.bypass, op1=ALU.add)
                    s_["u"] = u

                def emit_relu(c):
                    """hdr = bf16 relu(u) on DVE (4x); converts to f-major."""
                    s_ = st[c]
                    hdr = hdr_pool.tile([128, 4, CE], BF, name="hdr", tag="hdr")
                    for f in range(4):
                        nc.vector.tensor_scalar(
                            out=hdr[:, f, :].rearrange("p (e t) -> p e t", t=TB),
                            in0=s_["u"][:, :, f, :],
                            scalar1=0.0, scalar2=None, op0=ALU.max)
                    s_["hdr"] = hdr

                def _chain(dst, src, relu):
                    """dst[r] (+)= relu?(src[s-block rho-slice]) over 56 edges.

                    dst: [128, O, 4, TB]; src: [128, E, 4, TB]; flat slices.
                    First write per r-slot is non-accumulating.
                    """
                    W = 4 * TB
                    df = dst.rearrange("p o f t -> p (o f t)")
                    sf = src.rearrange("p e f t -> p (e f t)")

                    def op(dlo, dhi, elo, first):
                        n = dhi - dlo
                        s0 = sf[:, elo * W:(elo + n) * W]
                        d0 = df[:, dlo * W:dhi * W]
                        if relu:
                            if first:
                                nc.vector.tensor_scalar(
                                    out=d0, in0=s0,
                                    scalar1=0.0, scalar2=None, op0=ALU.max)
                            else:
                                nc.vector.scalar_tensor_tensor(
                                    out=d0, in0=s0, scalar=0.0, in1=d0,
                                    op0=ALU.max, op1=ALU.add)
                        else:
                            if first:
                                nc.vector.tensor_copy(d0, s0)
                            else:
                                nc.vector.scalar_tensor_tensor(
                                    out=d0, in0=s0, scalar=0.0, in1=d0,
                                    op0=ALU.bypass, op1=ALU.add)
                    # s = 0: slots rho 0..6 -> r 1..7 (first write)
                    op(1, O, 0, True)
                    # s = 1: rho 0 -> r 0 (first write), rho 1..6 -> r 2..7
                    op(0, 1, 7, True)
                    op(2, O, 8, False)
                    for s in range(2, O):
                        e0 = s * 7
                        op(0, s, e0, False)
                        if s < O - 1:
                            op(s + 1, O, e0 + s, False)

                def emit_z(c):
                    """msg = relu(h1 @ W2 + b2) in bf16 on PE + Act evicts."""
                    s_ = st[c]
                    hdr = s_["hdr"]
                    mst = mst_pool.tile([128, E, 4, TB], BF, name="mst", tag="mst")
                    # n-chunks of 512 cols (3 full + 1 half)
                    nspans = [(0, 512), (512, 1024), (1024, 1536), (1536, 1792)]
                    for mb in range(4):
                        for n0, n1 in nspans:
                            zt = zps.tile([128, 512], F32, name="zt", tag="z")
                            for k in range(4):
                                nc.tensor.matmul(
                                    zt[:, 0:n1 - n0],
                                    w2t[:, k, mb * 128:(mb + 1) * 128],
                                    hdr[:, k, n0:n1],
                                    start=(k == 0), stop=(k == 3))
                            dst = mst[:, n0 // TB:n1 // TB, mb, :]
                            zin = zt[:, 0:n1 - n0].rearrange(
                                "p (a t) -> p a t", t=TB)
                            nc.scalar.activation(dst, zin, AF.Relu,
                                                 bias=b2t[:, mb:mb + 1])
                    agg = agg_pool.tile([128, O, 4, TB], BF, name="agg", tag="agg")
                    _chain(agg, mst, relu=False)
                    s_["agg"] = agg

                def emit_head(c):
                    """out = inc^T Wn_in + agg^T Wn_agg + bn."""
                    s_ = st.pop(c)
                    t0 = c * TB
                    hp = opps.tile([64, NN], F32, name="hp", tag="op")
                    nc.tensor.matmul(hp[:], wnit[:], s_["inc"],
                                     start=True, stop=False)
                    for f in range(4):
                        nc.tensor.matmul(hp[:], wnat[:, f, :],
                                         s_["agg"][:, :, f, :],
                                         start=False, stop=(f == 3))
                    netoutT = net_pool.tile([64, NN], F32, name="netoutT")
                    nc.scalar.activation(netoutT[:], hp[:], AF.Identity,
                                         bias=bnt[:])
                    for hf in range(2):
                        tp2 = opps.tile([128, 64], F32, name="tp2", tag="op")
                        nc.tensor.transpose(
                            tp2[:], netoutT[:, hf * 128:(hf + 1) * 128],
                            ident[0:64, 0:64])
                        outrm = orm_pool.tile([128, 64], F32, name="outrm")
                        nc.vector.tensor_copy(outrm[:], tp2[:])
                        o0 = hf * 4
                        nc.sync.dma_start(out[o0:o0 + 4, t0:t0 + TB, :],
                                          outrm[:, :])

                for c in range(NCHUNK):
                    if c >= 1:
                        emit_relu(c - 1)
                    emit_ab(c)
                    emit_u(c)
                    if c >= 1:
                        emit_z(c - 1)
                    if c >= 2:
                        emit_head(c - 2)

                # ---- drain the software pipeline ----
                last = NCHUNK - 1
                emit_relu(last)
                emit_z(last)
                emit_head(last - 1)
                emit_head(last)

    nc.compile()
    return nc


_NC_CACHE = {}


def _get_nc():
    key = (MM_DT, 1)
    if key not in _NC_CACHE:
        _NC_CACHE[key] = build_nc(MM_DT, 1)
    return _NC_CACHE[key]


def shard_inputs(x, forward_probs, **_):
    x = np.ascontiguousarray(np.asarray(x, dtype=np.float32))
    fp = np.ascontiguousarray(np.asarray(forward_probs, dtype=np.float32))
    in_maps = []
    for c in range(8):
        b, th = c // 2, c % 2
        in_maps.append({
            "xs": np.ascontiguousarray(x[b, :, th * TC:(th + 1) * TC, :]),
            "fps": np.ascontiguousarray(fp[b, :, th * TC:(th + 1) * TC, :]),
        })
    return in_maps


def prep_weights(W1, b1, W2, b2, Wn, bn):
    W1 = np.asarray(W1, np.float32)
    b1 = np.asarray(b1, np.float32)
    W2 = np.asarray(W2, np.float32)
    b2 = np.asarray(b2, np.float32)
    Wn = np.asarray(Wn, np.float32)
    bn = np.asarray(bn, np.float32)
    return {
        "w1": np.ascontiguousarray(W1),
        "b1": b1,
        "w2b": np.ascontiguousarray(W2.astype(ml_dtypes.bfloat16)),
        "b2": b2,
        "wnin": np.ascontiguousarray(Wn[:D]),
        "wna": np.ascontiguousarray(Wn[D:].astype(ml_dtypes.bfloat16)),
        "bn": bn,
    }


def kernel(y, x, hidden_states, forward_probs, edge_est, edge_gt,
           W1, b1, W2, b2, Wn, bn, edge2node):
    nc = _get_nc()
    weights = prep_weights(W1, b1, W2, b2, Wn, bn)
    in_maps = [dict(m, **weights) for m in shard_inputs(x, forward_probs)]
    res = run_bass_kernel_spmd(nc, in_maps, list(range(8)))
    full = np.empty((B, O, T, KK), dtype=np.float32)
    for c in range(8):
        b, th = c // 2, c % 2
        full[b, :, th * TC:(th + 1) * TC, :] = res.results[c]["out"]
    return full.reshape(B, O, T, 8, 8)
